# revision 10
# baseline (speedup 1.0000x reference)
"""Trainium2 Bass kernel for a 6-layer causal decoder transformer (v2).

Model: B=128, T=256, E=384, H=6, D=64, DFF=1536, L=6, V=65 (f32 reference).
Sharding: pure data-parallel over batch across 8 NeuronCores (16 batches
per core), parameters replicated, no collectives.

v2 changes vs v1 (engine-balance redesign, from CoreSim cost analysis):
  - Scores computed TRANSPOSED on PE (scoresT[k,q] = k^T q) so the exp'd
    probabilities are already in the layout the AV matmul needs as lhsT —
    the 3 per-(batch,head) P-transposes and their PSUM copy-outs are gone.
  - One [128,384] scoresT PSUM tile per (batch,head): ktile0 x q(0:256) in
    cols 0:256, ktile1 x q(128:256) in cols 256:384; causal mask added by
    PE (identity @ maskT accumulate) on the two diagonal blocks; ONE exp
    activation per head (no accum_out — softmax denominators come from
    N=1 ones-column matmuls accumulated into the AV PSUM tile).
  - AV output token-major [q, d] (N=64 matmuls), normalized during the
    PSUM->SBUF copy by per-partition reciprocal scaling; o then transposed
    to feature-major for the Wo matmul (3 PE transposes + ONE batched copy
    per token tile).
  - LayerNorm x-hat apply runs on the otherwise-idle GpSimd engine.
  - Transpose copy-outs batched ([P,3,128] PSUM -> one strided copy).
  - PSUM->SBUF copy-outs hand-balanced across ScalarE/VectorE.
"""

import sys
from contextlib import ExitStack

sys.path.insert(0, "/opt/trn_rl_repo")

import numpy as np
import ml_dtypes

import concourse.bass as bass
import concourse.bacc as bacc
import concourse.mybir as mybir
import concourse.tile as tile
from concourse.masks import make_identity
from concourse.bass_utils import run_bass_kernel_spmd

F32 = mybir.dt.float32
BF16 = mybir.dt.bfloat16
AF = mybir.ActivationFunctionType
OP = mybir.AluOpType

P = 128
E, DFF, H, D, T, L, V = 384, 1536, 6, 64, 256, 6, 65
B = 128
N_CORES = 8
B_LOC = B // N_CORES          # 16 batches per core
NTOK = B_LOC * T              # 4096 tokens per core
NT = NTOK // P                # 32 token tiles
GROUP = 512                   # tokens per group (2 full batches)
NG = NTOK // GROUP            # 8 groups
TPG = GROUP // P              # 4 token tiles per group
BPG = GROUP // T              # 2 batches per group
EC = E // P                   # 3 feature chunks
FC = DFF // P                 # 12 dff chunks
NEG = -1.0e9

_PROG = None  # (nc, zero_bias)


def _ln_stats_group(nc, stat, x_list, eps=1e-5):
    """bn_stats per tile + batched Newton rsqrt. Returns (mv_g, rs_g):
    mv_g[:, i, 0:1] = mean of tile i; rs_g[:, i:i+1] = rsqrt(var_i + eps)."""
    n = len(x_list)
    mv_g = stat.tile([P, n, 2], F32, tag="mvg")
    for i, xin in enumerate(x_list):
        st6 = stat.tile([P, 6], F32, tag="bn6")
        nc.vector.bn_stats(out=st6[:], in_=xin)
        nc.vector.bn_aggr(out=mv_g[:, i, :], in_=st6[:])
    var = stat.tile([P, n], F32, tag="vare")
    nc.vector.tensor_scalar_add(var[:], mv_g[:, :, 1], eps)
    u = stat.tile([P, n], F32, tag="ue")
    nc.vector.reciprocal(u[:], var[:])
    lin = stat.tile([P, n], F32, tag="line")
    nc.vector.tensor_scalar(lin[:], var[:], 0.73, 0.32, op0=OP.mult, op1=OP.add)
    rs = stat.tile([P, n], F32, tag="rse")
    nc.vector.tensor_tensor(rs[:], u[:], lin[:], OP.mult)       # seed ~ rsqrt
    t1 = stat.tile([P, n], F32, tag="t1e")
    for _ in range(2):                                          # Newton x2
        nc.vector.tensor_tensor(t1[:], rs[:], rs[:], OP.mult)
        nc.vector.tensor_tensor(t1[:], t1[:], var[:], OP.mult)
        nc.vector.tensor_scalar(t1[:], t1[:], -0.5, 1.5, op0=OP.mult, op1=OP.add)
        nc.vector.tensor_tensor(rs[:], rs[:], t1[:], OP.mult)
    return mv_g, rs


def build_program(repeat=1, dma_t=False, zero_bias=True):
    nc = bacc.Bacc("TRN2", target_bir_lowering=False, debug=False,
                   num_devices=N_CORES)

    # register const APs needed for float biases on ScalarE activations
    for val in (1e-5,):
        t = nc.alloc_sbuf_tensor(f"const-f32-{val}", [P, 1], F32)
        nc.gpsimd.memset(t.ap(), val)
        nc.const_aps.aps[(F32, val)] = t.ap()
    nc.all_engine_barrier()

    # ---- I/O -------------------------------------------------------------
    oht = nc.dram_tensor("oht", [P, NTOK], BF16, kind="ExternalInput").ap()
    embp = nc.dram_tensor("embp", [P, E], BF16, kind="ExternalInput").ap()
    pose = nc.dram_tensor("pose", [T, E], F32, kind="ExternalInput").ap()
    maskt = nc.dram_tensor("maskt", [P, P], BF16, kind="ExternalInput").ap()
    wq = nc.dram_tensor("wq", [L, E, E], BF16, kind="ExternalInput").ap()
    wk = nc.dram_tensor("wk", [L, E, E], BF16, kind="ExternalInput").ap()
    wv = nc.dram_tensor("wv", [L, E, E], BF16, kind="ExternalInput").ap()
    wo = nc.dram_tensor("wo", [L, E, E], BF16, kind="ExternalInput").ap()
    w1 = nc.dram_tensor("w1", [L, E, DFF], BF16, kind="ExternalInput").ap()
    w2 = nc.dram_tensor("w2", [L, DFF, E], BF16, kind="ExternalInput").ap()
    wl = nc.dram_tensor("wl", [E, V], BF16, kind="ExternalInput").ap()
    bqf = nc.dram_tensor("bqf", [L, P, EC], F32, kind="ExternalInput").ap()
    bkf = nc.dram_tensor("bkf", [L, P, EC], F32, kind="ExternalInput").ap()
    c1f = nc.dram_tensor("c1f", [L, P, FC], F32, kind="ExternalInput").ap()
    btm = nc.dram_tensor("btm", [L, 3, P, E], F32, kind="ExternalInput").ap()
    blr = nc.dram_tensor("blr", [P, V], F32, kind="ExternalInput").ap()
    out = nc.dram_tensor("out", [NTOK, V], F32, kind="ExternalOutput").ap()

    with tile.TileContext(nc) as tc, ExitStack() as es:
            ep = es.enter_context
            const = ep(tc.tile_pool(name="const", bufs=1))
            xres = ep(tc.tile_pool(name="xres", bufs=1))
            wa = ep(tc.tile_pool(name="wa", bufs=2))
            wf = ep(tc.tile_pool(name="wf", bufs=2))
            bias = ep(tc.tile_pool(name="bias", bufs=2))
            grp = ep(tc.tile_pool(name="grp", bufs=2))
            grp1 = ep(tc.tile_pool(name="grp1", bufs=1))
            vt = ep(tc.tile_pool(name="vt", bufs=6))
            tk = ep(tc.tile_pool(name="tk", bufs=4))
            ptp = ep(tc.tile_pool(name="ptp", bufs=6))
            otp = ep(tc.tile_pool(name="otp", bufs=2))
            stat = ep(tc.tile_pool(name="stat", bufs=8))
            psmm = ep(tc.tile_pool(name="psmm", bufs=2, space="PSUM"))
            pss = ep(tc.tile_pool(name="pss", bufs=2, space="PSUM"))
            psav = ep(tc.tile_pool(name="psav", bufs=2, space="PSUM"))
            pstr = ep(tc.tile_pool(name="pstr", bufs=2, space="PSUM"))
            # ---- constants ----
            id_bf = const.tile([P, P], BF16, tag="id_bf")
            make_identity(nc, id_bf)
            mask_sb = const.tile([P, P], BF16, tag="mask")
            nc.sync.dma_start(mask_sb[:], maskt[:])
            ones_col = const.tile([P, 1], BF16, tag="ones")
            nc.vector.memset(ones_col[:], 1.0)
            emb_sb = const.tile([P, E], BF16, tag="emb")
            nc.sync.dma_start(emb_sb[:], embp[:])
            pose_sb = const.tile([P, 2, E], F32, tag="pose")
            nc.sync.dma_start(pose_sb[:, 0, :], pose[0:P, :])
            nc.sync.dma_start(pose_sb[:, 1, :], pose[P : 2 * P, :])
            wl_sb = const.tile([P, EC, V], BF16, tag="wl")
            nc.sync.dma_start(wl_sb[:], wl.rearrange("(kc p) n -> p kc n", p=P))
            blr_sb = const.tile([P, V], F32, tag="blr")
            nc.sync.dma_start(blr_sb[:], blr[:])
            oht_sb = const.tile([P, NTOK], BF16, tag="oht")
            nc.sync.dma_start(oht_sb[:], oht[:])

            x_tm = [xres.tile([P, E], F32, tag=f"x{t}", name=f"x{t}") for t in range(NT)]

            NU = L * NG                  # pipelined (layer, group) units
            weights = {}                 # l -> weight/bias tiles
            st = {}                      # u -> inter-stage tiles

            def load_weights(l):
                w = {}
                w["wq"] = wa.tile([P, EC, E], BF16, tag="wq", name="wq_sb")
                nc.sync.dma_start(w["wq"][:], wq[l].rearrange("(kc p) n -> p kc n", p=P))
                w["wk"] = wa.tile([P, EC, E], BF16, tag="wk", name="wk_sb")
                nc.sync.dma_start(w["wk"][:], wk[l].rearrange("(kc p) n -> p kc n", p=P))
                w["wv"] = wa.tile([P, EC, E], BF16, tag="wv", name="wv_sb")
                nc.sync.dma_start(w["wv"][:], wv[l].rearrange("(kc p) n -> p kc n", p=P))
                w["wo"] = wa.tile([P, EC, E], BF16, tag="wo", name="wo_sb")
                nc.sync.dma_start(w["wo"][:], wo[l].rearrange("(kc p) n -> p kc n", p=P))
                w["w1"] = wf.tile([P, EC, DFF], BF16, tag="w1", name="w1_sb")
                nc.sync.dma_start(w["w1"][:], w1[l].rearrange("(kc p) n -> p kc n", p=P))
                w["w2"] = wf.tile([P, FC, E], BF16, tag="w2", name="w2_sb")
                nc.sync.dma_start(w["w2"][:], w2[l].rearrange("(kc p) n -> p kc n", p=P))
                if not zero_bias:
                    w["bq"] = bias.tile([P, EC], F32, tag="bq", name="bq_sb")
                    nc.sync.dma_start(w["bq"][:], bqf[l])
                    w["bk"] = bias.tile([P, EC], F32, tag="bk", name="bk_sb")
                    nc.sync.dma_start(w["bk"][:], bkf[l])
                    w["c1"] = bias.tile([P, FC], F32, tag="c1", name="c1_sb")
                    nc.sync.dma_start(w["c1"][:], c1f[l])
                    w["btm"] = bias.tile([P, 3, E], F32, tag="btm", name="btm_sb")
                    nc.sync.dma_start(w["btm"][:], btm[l].rearrange("t p n -> p t n"))
                weights[l] = w

            def stageA(u):
                """LN1 + QKV projections + attention -> o_tm tiles."""
                l, g = u // NG, u % NG
                w = weights[l]
                tts = [g * TPG + i for i in range(TPG)]

                h_fm = grp.tile([P, EC, GROUP], BF16, tag="hfm", name="h_fm")
                mv_g, rs_g = _ln_stats_group(nc, stat, [x_tm[tt][:] for tt in tts])
                for i, tt in enumerate(tts):
                    xh = tk.tile([P, E], BF16, tag="xh1", name="xh")
                    nc.gpsimd.tensor_scalar(xh[:], x_tm[tt][:],
                                            mv_g[:, i, 0:1], rs_g[:, i : i + 1],
                                            op0=OP.subtract, op1=OP.mult)
                    ptt = pstr.tile([P, EC, P], BF16, tag="tr", name="ptt")
                    for kc in range(EC):
                        nc.tensor.transpose(ptt[:, kc, :],
                                            xh[:, kc * P : (kc + 1) * P], id_bf[:])
                    if i % 2 == 0:
                        nc.vector.tensor_copy(h_fm[:, :, i * P : (i + 1) * P], ptt[:])
                    else:
                        nc.scalar.copy(h_fm[:, :, i * P : (i + 1) * P], ptt[:])

                q_fm = grp.tile([P, EC, GROUP], BF16, tag="qfm", name="q_fm")
                k_fm = grp.tile([P, EC, GROUP], BF16, tag="kfm", name="k_fm")
                for dst, wsb, bname in ((q_fm, w["wq"], "bq"), (k_fm, w["wk"], "bk")):
                    for m in range(EC):
                        pq = psmm.tile([P, GROUP], F32, tag="mm", name="pq")
                        for kc in range(EC):
                            nc.tensor.matmul(pq[:], wsb[:, kc, m * P : (m + 1) * P],
                                             h_fm[:, kc, :],
                                             start=(kc == 0), stop=(kc == EC - 1))
                        if zero_bias:
                            nc.scalar.copy(dst[:, m, :], pq[:])
                        else:
                            nc.scalar.activation(dst[:, m, :], pq[:], AF.Identity,
                                                 bias=w[bname][:, m : m + 1], scale=1.0)

                v_tiles = []
                for i, tt in enumerate(tts):
                    pv = psmm.tile([P, GROUP], F32, tag="mm", name="pv")
                    for kc in range(EC):
                        nc.tensor.matmul(pv[:, :E], h_fm[:, kc, i * P : (i + 1) * P],
                                         w["wv"][:, kc, :],
                                         start=(kc == 0), stop=(kc == EC - 1))
                    vt_i = vt.tile([P, E], BF16, tag="vtm", name="vt_i")
                    if zero_bias:
                        nc.vector.tensor_copy(vt_i[:], pv[:, :E])
                    else:
                        nc.vector.tensor_tensor(vt_i[:], pv[:, :E], w["btm"][:, 0, :], OP.add)
                    v_tiles.append(vt_i)

                o_tm = [otp.tile([P, E], BF16, tag=f"otm{i}", name=f"otm{i}")
                        for i in range(TPG)]
                for lb in range(BPG):
                    v0 = v_tiles[2 * lb]
                    v1 = v_tiles[2 * lb + 1]
                    q0 = lb * T
                    for j in range(EC):          # head pair (2j, 2j+1)
                        pav = psav.tile([P, 260], F32, tag="av", name="pav")
                        for hh in range(2):
                            h = 2 * j + hh
                            ro = (h % 2) * 64
                            mc = h // 2
                            q_ap = q_fm[ro : ro + 64, mc, q0 : q0 + T]
                            k_ap = k_fm[ro : ro + 64, mc, q0 : q0 + T]

                            # scoresT [k, q]: cols 0:256 = ktile0 x q(0:256),
                            # cols 256:384 = ktile1 x q(128:256)
                            ps = pss.tile([P, E], F32, tag="s", name="ps")
                            nc.tensor.matmul(ps[:, 0:T], k_ap[:, 0:P], q_ap[:],
                                             start=True, stop=False)
                            nc.tensor.matmul(ps[:, 0:P], id_bf[:], mask_sb[:],
                                             start=False, stop=True)
                            nc.tensor.matmul(ps[:, T:E], k_ap[:, P:T], q_ap[:, P:T],
                                             start=True, stop=False)
                            nc.tensor.matmul(ps[:, T:E], id_bf[:], mask_sb[:],
                                             start=False, stop=True)
                            pt_sb = ptp.tile([P, E], BF16, tag="pt", name="pt_sb")
                            nc.scalar.activation(pt_sb[:], ps[:], AF.Exp,
                                                 bias=0.0, scale=1.0)

                            # AV token-major + ones-column sums
                            vsl = slice(h * 64, (h + 1) * 64)
                            c = hh * 64
                            sc = 256 + 2 * hh
                            nc.tensor.matmul(pav[:, c : c + 64], pt_sb[:, 0:P],
                                             v0[:, vsl], start=True, stop=True)
                            nc.tensor.matmul(pav[:, sc : sc + 1], pt_sb[:, 0:P],
                                             ones_col[:], start=True, stop=True)
                            nc.tensor.matmul(pav[:, 128 + c : 128 + c + 64],
                                             pt_sb[:, P:T], v0[:, vsl],
                                             start=True, stop=False)
                            nc.tensor.matmul(pav[:, 128 + c : 128 + c + 64],
                                             pt_sb[:, T:E], v1[:, vsl],
                                             start=False, stop=True)
                            nc.tensor.matmul(pav[:, sc + 1 : sc + 2],
                                             pt_sb[:, P:T], ones_col[:],
                                             start=True, stop=False)
                            nc.tensor.matmul(pav[:, sc + 1 : sc + 2],
                                             pt_sb[:, T:E], ones_col[:],
                                             start=False, stop=True)

                        # normalize on copy-out: cols 256:260 hold
                        # [s_h0_q0, s_h0_q1, s_h1_q0, s_h1_q1]
                        rs4 = stat.tile([P, 4], F32, tag="rs4", name="rs4")
                        nc.vector.reciprocal(rs4[:], pav[:, 256:260])
                        # (recip col, pav col, q-tile within pair)
                        for idx, (ri, pc, qi) in enumerate(
                                ((0, 0, 0), (1, 128, 1), (2, 64, 0), (3, 192, 1))):
                            htarget = 2 * j + (0 if idx < 2 else 1)
                            dst = o_tm[2 * lb + qi][:, htarget * 64 : htarget * 64 + 64]
                            if idx % 2 == 0:
                                nc.vector.tensor_scalar_mul(
                                    dst, pav[:, pc : pc + 64], rs4[:, ri : ri + 1])
                            else:
                                nc.scalar.activation(
                                    dst, pav[:, pc : pc + 64], AF.Copy,
                                    bias=0.0, scale=rs4[:, ri : ri + 1])
                st[u] = {"o_tm": o_tm}

            def stageB1(u):
                """o transpose + Wo + residual + LN2 stats/apply -> xh2 tiles."""
                l, g = u // NG, u % NG
                w = weights[l]
                tts = [g * TPG + i for i in range(TPG)]
                o_tm = st[u]["o_tm"]

                o_fm = grp.tile([P, EC, GROUP], BF16, tag="ofm", name="o_fm")
                for i in range(TPG):
                    pto = pstr.tile([P, EC, P], BF16, tag="tr", name="pto")
                    for kc in range(EC):
                        nc.tensor.transpose(pto[:, kc, :],
                                            o_tm[i][:, kc * P : (kc + 1) * P], id_bf[:])
                    if i % 2 == 0:
                        nc.scalar.copy(o_fm[:, :, i * P : (i + 1) * P], pto[:])
                    else:
                        nc.vector.tensor_copy(o_fm[:, :, i * P : (i + 1) * P], pto[:])

                for i, tt in enumerate(tts):
                    pao = psmm.tile([P, GROUP], F32, tag="mm", name="pao")
                    for kc in range(EC):
                        nc.tensor.matmul(pao[:, :E], o_fm[:, kc, i * P : (i + 1) * P],
                                         w["wo"][:, kc, :],
                                         start=(kc == 0), stop=(kc == EC - 1))
                    if zero_bias:
                        nc.vector.tensor_tensor(x_tm[tt][:], pao[:, :E], x_tm[tt][:], OP.add)
                    else:
                        t1 = tk.tile([P, E], F32, tag="t1", name="t1")
                        nc.vector.tensor_tensor(t1[:], pao[:, :E], x_tm[tt][:], OP.add)
                        nc.gpsimd.tensor_tensor(x_tm[tt][:], t1[:], w["btm"][:, 1, :], OP.add)

                mv_g2, rs_g2 = _ln_stats_group(nc, stat, [x_tm[tt][:] for tt in tts])
                xh2s = []
                for i, tt in enumerate(tts):
                    xh2 = tk.tile([P, E], BF16, tag="xh2", name="xh2")
                    nc.gpsimd.tensor_scalar(xh2[:], x_tm[tt][:],
                                            mv_g2[:, i, 0:1], rs_g2[:, i : i + 1],
                                            op0=OP.subtract, op1=OP.mult)
                    xh2s.append(xh2)
                st[u]["xh2"] = xh2s

            def stageB2(u):
                """LN2 transposes -> h2_fm."""
                h2_fm = grp.tile([P, EC, GROUP], BF16, tag="h2fm", name="h2_fm")
                for i, xh2 in enumerate(st[u]["xh2"]):
                    ptt2 = pstr.tile([P, EC, P], BF16, tag="tr", name="ptt2")
                    for kc in range(EC):
                        nc.tensor.transpose(ptt2[:, kc, :],
                                            xh2[:, kc * P : (kc + 1) * P], id_bf[:])
                    if i % 2 == 0:
                        nc.scalar.copy(h2_fm[:, :, i * P : (i + 1) * P], ptt2[:])
                    else:
                        nc.vector.tensor_copy(h2_fm[:, :, i * P : (i + 1) * P], ptt2[:])
                st[u]["h2_fm"] = h2_fm

            def stageC(u):
                """FFN: W1+relu, W2+residual."""
                l, g = u // NG, u % NG
                w = weights[l]
                tts = [g * TPG + i for i in range(TPG)]
                h2_fm = st[u]["h2_fm"]

                hf = grp1.tile([P, FC, GROUP], BF16, tag="hf", name="hf")
                for m in range(FC):
                    pf = psmm.tile([P, GROUP], F32, tag="mm", name="pf")
                    for kc in range(EC):
                        nc.tensor.matmul(pf[:], w["w1"][:, kc, m * P : (m + 1) * P],
                                         h2_fm[:, kc, :],
                                         start=(kc == 0), stop=(kc == EC - 1))
                    if zero_bias:
                        if m % 3 == 2:
                            nc.vector.tensor_scalar_max(hf[:, m, :], pf[:], 0.0)
                        else:
                            nc.scalar.activation(hf[:, m, :], pf[:], AF.Relu,
                                                 bias=0.0, scale=1.0)
                    else:
                        if m % 3 == 2:
                            nc.vector.tensor_scalar(hf[:, m, :], pf[:],
                                                    w["c1"][:, m : m + 1], 0.0,
                                                    op0=OP.add, op1=OP.max)
                        else:
                            nc.scalar.activation(hf[:, m, :], pf[:], AF.Relu,
                                                 bias=w["c1"][:, m : m + 1], scale=1.0)

                for i, tt in enumerate(tts):
                    pw2 = psmm.tile([P, GROUP], F32, tag="mm", name="pw2")
                    for kc in range(FC):
                        nc.tensor.matmul(pw2[:, :E], hf[:, kc, i * P : (i + 1) * P],
                                         w["w2"][:, kc, :],
                                         start=(kc == 0), stop=(kc == FC - 1))
                    if zero_bias:
                        nc.vector.tensor_tensor(x_tm[tt][:], pw2[:, :E], x_tm[tt][:], OP.add)
                    else:
                        t2 = tk.tile([P, E], F32, tag="t1", name="t2")
                        nc.vector.tensor_tensor(t2[:], pw2[:, :E], x_tm[tt][:], OP.add)
                        nc.gpsimd.tensor_tensor(x_tm[tt][:], t2[:], w["btm"][:, 2, :], OP.add)
                del st[u]

            for _rep in range(repeat):
                # ---- x0 = onehot @ emb + pos ----
                for tt in range(NT):
                    xt = x_tm[tt]
                    pe = psmm.tile([P, GROUP], F32, tag="mm", name="pe")
                    nc.tensor.matmul(pe[:, :E], oht_sb[:, tt * P : (tt + 1) * P],
                                     emb_sb[:], start=True, stop=True)
                    nc.vector.tensor_tensor(xt[:], pe[:, :E], pose_sb[:, tt % 2, :], OP.add)

                # ---- pipelined layers: A(u) | B1(u-1) | B2(u-2), C(u-2) ----
                load_weights(0)
                for u in range(NU):
                    l, g = u // NG, u % NG
                    if g == 2 and l + 1 < L:
                        load_weights(l + 1)
                    stageA(u)
                    if u >= 1:
                        stageB1(u - 1)
                    if u >= 2:
                        stageB2(u - 2)
                        stageC(u - 2)
                stageB1(NU - 1)
                stageB2(NU - 2)
                stageC(NU - 2)
                stageB2(NU - 1)
                stageC(NU - 1)

                # ---- final logits ----
                for tt in range(NT):
                    xb = tk.tile([P, E], BF16, tag="xhat")
                    nc.any.tensor_copy(out=xb[:], in_=x_tm[tt][:])
                    ptl = pstr.tile([P, EC, P], BF16, tag="tr")
                    for kc in range(EC):
                        nc.tensor.transpose(ptl[:, kc, :],
                                            xb[:, kc * P : (kc + 1) * P], id_bf[:])
                    xf = tk.tile([P, EC, P], BF16, tag="xf")
                    if tt % 2 == 0:
                        nc.vector.tensor_copy(xf[:], ptl[:])
                    else:
                        nc.scalar.copy(xf[:], ptl[:])
                    pl = psmm.tile([P, GROUP], F32, tag="mm")
                    for kc in range(EC):
                        nc.tensor.matmul(pl[:, :V], xf[:, kc, :], wl_sb[:, kc, :],
                                         start=(kc == 0), stop=(kc == EC - 1))
                    lg = tk.tile([P, V], F32, tag="lg")
                    if zero_bias:
                        if tt % 2 == 0:
                            nc.scalar.copy(lg[:], pl[:, :V])
                        else:
                            nc.vector.tensor_copy(lg[:], pl[:, :V])
                    else:
                        nc.vector.tensor_tensor(lg[:], pl[:, :V], blr_sb[:], OP.add)
                    nc.sync.dma_start(out[tt * P : (tt + 1) * P, :], lg[:])

    nc.compile()
    return nc


def _prep_host(inputs):
    f32 = np.float32
    bf16 = ml_dtypes.bfloat16
    tokens = np.asarray(inputs["tokens"]).astype(np.int64)
    emb = np.asarray(inputs["emb"], dtype=f32)
    pos_enc = np.asarray(inputs["pos_enc"], dtype=f32)
    Wq = np.asarray(inputs["Wq"], dtype=f32)
    Wk = np.asarray(inputs["Wk"], dtype=f32)
    Wv = np.asarray(inputs["Wv"], dtype=f32)
    Wo = np.asarray(inputs["Wo"], dtype=f32)
    W1 = np.asarray(inputs["W1"], dtype=f32)
    W2 = np.asarray(inputs["W2"], dtype=f32)
    Wl = np.asarray(inputs["Wl"], dtype=f32)
    bq = np.asarray(inputs["bq"], dtype=f32)
    bk = np.asarray(inputs["bk"], dtype=f32)
    bv = np.asarray(inputs["bv"], dtype=f32)
    bo = np.asarray(inputs["bo"], dtype=f32)
    c1 = np.asarray(inputs["c1"], dtype=f32)
    c2 = np.asarray(inputs["c2"], dtype=f32)
    bl = np.asarray(inputs["bl"], dtype=f32)
    g1 = np.asarray(inputs["ln1_g"], dtype=f32)
    b1 = np.asarray(inputs["ln1_b"], dtype=f32)
    g2 = np.asarray(inputs["ln2_g"], dtype=f32)
    b2 = np.asarray(inputs["ln2_b"], dtype=f32)

    scale = D ** -0.5
    wq_f = np.empty((L, E, E), f32)
    wk_f = np.empty((L, E, E), f32)
    wv_f = np.empty((L, E, E), f32)
    w1_f = np.empty((L, E, DFF), f32)
    bq_f = np.empty((L, E), f32)
    bk_f = np.empty((L, E), f32)
    bv_f = np.empty((L, E), f32)
    c1_f = np.empty((L, DFF), f32)
    for l in range(L):
        wq_f[l] = g1[l][:, None] * Wq[l] * scale
        bq_f[l] = (b1[l] @ Wq[l] + bq[l]) * scale
        wk_f[l] = g1[l][:, None] * Wk[l]
        bk_f[l] = b1[l] @ Wk[l] + bk[l]
        wv_f[l] = g1[l][:, None] * Wv[l]
        bv_f[l] = b1[l] @ Wv[l] + bv[l]
        w1_f[l] = g2[l][:, None] * W1[l]
        c1_f[l] = b2[l] @ W1[l] + c1[l]

    # maskt[k, q] = 0 if k <= q else NEG  (transposed causal mask)
    maskt = np.where(np.tril(np.ones((P, P), bool)).T, 0.0, NEG).astype(bf16)

    common = {
        "embp": np.zeros((P, E), bf16),
        "pose": pos_enc,
        "maskt": maskt,
        "wq": wq_f.astype(bf16),
        "wk": wk_f.astype(bf16),
        "wv": wv_f.astype(bf16),
        "wo": Wo.astype(bf16),
        "w1": w1_f.astype(bf16),
        "w2": W2.astype(bf16),
        "wl": Wl.astype(bf16),
        "bqf": np.ascontiguousarray(bq_f.reshape(L, EC, P).transpose(0, 2, 1)),
        "bkf": np.ascontiguousarray(bk_f.reshape(L, EC, P).transpose(0, 2, 1)),
        "c1f": np.ascontiguousarray(c1_f.reshape(L, FC, P).transpose(0, 2, 1)),
        "btm": np.ascontiguousarray(
            np.broadcast_to(
                np.stack([bv_f, bo, c2], axis=1)[:, :, None, :], (L, 3, P, E)
            )
        ).astype(f32),
        "blr": np.broadcast_to(bl[None, :], (P, V)).astype(f32),
    }
    common["embp"][:V, :] = emb.astype(bf16)

    in_maps = []
    for c in range(N_CORES):
        tok_c = tokens[c * B_LOC : (c + 1) * B_LOC].reshape(-1)
        oht = np.zeros((P, NTOK), bf16)
        oht[tok_c, np.arange(NTOK)] = 1
        m = dict(common)
        m["oht"] = oht
        in_maps.append(m)
    return in_maps


def _biases_all_zero(inputs):
    zs = [inputs[k] for k in ("bq", "bk", "bv", "bo", "c1", "c2", "bl",
                              "ln1_b", "ln2_b")]
    return all(not np.any(np.asarray(z)) for z in zs)


def kernel(**inputs) -> np.ndarray:
    global _PROG
    zb = _biases_all_zero(inputs)
    if _PROG is None or _PROG[1] != zb:
        _PROG = (build_program(zero_bias=zb), zb)
    nc = _PROG[0]
    in_maps = _prep_host(inputs)
    res = run_bass_kernel_spmd(nc, in_maps, list(range(N_CORES)))
    outs = [res.results[c]["out"].reshape(B_LOC, T, V) for c in range(N_CORES)]
    return np.concatenate(outs, axis=0).astype(np.float32)


# revision 12
# speedup vs baseline: 1.9264x; 1.9264x over previous
"""Trainium2 Bass kernel for a 6-layer causal decoder transformer (v2).

Model: B=128, T=256, E=384, H=6, D=64, DFF=1536, L=6, V=65 (f32 reference).
Sharding: pure data-parallel over batch across 8 NeuronCores (16 batches
per core), parameters replicated, no collectives.

v2 changes vs v1 (engine-balance redesign, from CoreSim cost analysis):
  - Scores computed TRANSPOSED on PE (scoresT[k,q] = k^T q) so the exp'd
    probabilities are already in the layout the AV matmul needs as lhsT —
    the 3 per-(batch,head) P-transposes and their PSUM copy-outs are gone.
  - One [128,384] scoresT PSUM tile per (batch,head): ktile0 x q(0:256) in
    cols 0:256, ktile1 x q(128:256) in cols 256:384; causal mask added by
    PE (identity @ maskT accumulate) on the two diagonal blocks; ONE exp
    activation per head (no accum_out — softmax denominators come from
    N=1 ones-column matmuls accumulated into the AV PSUM tile).
  - AV output token-major [q, d] (N=64 matmuls), normalized during the
    PSUM->SBUF copy by per-partition reciprocal scaling; o then transposed
    to feature-major for the Wo matmul (3 PE transposes + ONE batched copy
    per token tile).
  - LayerNorm x-hat apply runs on the otherwise-idle GpSimd engine.
  - Transpose copy-outs batched ([P,3,128] PSUM -> one strided copy).
  - PSUM->SBUF copy-outs hand-balanced across ScalarE/VectorE.
"""

import sys
from contextlib import ExitStack

sys.path.insert(0, "/opt/trn_rl_repo")

import numpy as np
import ml_dtypes

import concourse.bass as bass
import concourse.bacc as bacc
import concourse.mybir as mybir
import concourse.tile as tile
from concourse.masks import make_identity
from concourse.bass_utils import run_bass_kernel_spmd

F32 = mybir.dt.float32
BF16 = mybir.dt.bfloat16
AF = mybir.ActivationFunctionType
OP = mybir.AluOpType

P = 128
E, DFF, H, D, T, L, V = 384, 1536, 6, 64, 256, 6, 65
B = 128
N_CORES = 8
B_LOC = B // N_CORES          # 16 batches per core
NTOK = B_LOC * T              # 4096 tokens per core
NT = NTOK // P                # 32 token tiles
GROUP = 512                   # tokens per group (2 full batches)
NG = NTOK // GROUP            # 8 groups
TPG = GROUP // P              # 4 token tiles per group
BPG = GROUP // T              # 2 batches per group
EC = E // P                   # 3 feature chunks
FC = DFF // P                 # 12 dff chunks
NEG = -1.0e9

_PROG = None  # (nc, zero_bias)
LN_ENG = lambda nc: nc.vector  # engine for LN x-hat apply (A/B testable)


def _ln_stats_group(nc, stat, x_list, eps=1e-5):
    """bn_stats per tile + batched Newton rsqrt. Returns (mv_g, rs_g):
    mv_g[:, i, 0:1] = mean of tile i; rs_g[:, i:i+1] = rsqrt(var_i + eps)."""
    n = len(x_list)
    mv_g = stat.tile([P, n, 2], F32, tag="mvg")
    for i, xin in enumerate(x_list):
        st6 = stat.tile([P, 6], F32, tag="bn6")
        nc.vector.bn_stats(out=st6[:], in_=xin)
        nc.vector.bn_aggr(out=mv_g[:, i, :], in_=st6[:])
    var = stat.tile([P, n], F32, tag="vare")
    nc.vector.tensor_scalar_add(var[:], mv_g[:, :, 1], eps)
    u = stat.tile([P, n], F32, tag="ue")
    nc.vector.reciprocal(u[:], var[:])
    lin = stat.tile([P, n], F32, tag="line")
    nc.vector.tensor_scalar(lin[:], var[:], 0.73, 0.32, op0=OP.mult, op1=OP.add)
    rs = stat.tile([P, n], F32, tag="rse")
    nc.vector.tensor_tensor(rs[:], u[:], lin[:], OP.mult)       # seed ~ rsqrt
    t1 = stat.tile([P, n], F32, tag="t1e")
    for _ in range(2):                                          # Newton x2
        nc.vector.tensor_tensor(t1[:], rs[:], rs[:], OP.mult)
        nc.vector.tensor_tensor(t1[:], t1[:], var[:], OP.mult)
        nc.vector.tensor_scalar(t1[:], t1[:], -0.5, 1.5, op0=OP.mult, op1=OP.add)
        nc.vector.tensor_tensor(rs[:], rs[:], t1[:], OP.mult)
    return mv_g, rs


def build_program(repeat=1, dma_t=False, zero_bias=True):
    nc = bacc.Bacc("TRN2", target_bir_lowering=False, debug=False,
                   num_devices=N_CORES)

    # register const APs needed for float biases on ScalarE activations
    for val in (1e-5,):
        t = nc.alloc_sbuf_tensor(f"const-f32-{val}", [P, 1], F32)
        nc.gpsimd.memset(t.ap(), val)
        nc.const_aps.aps[(F32, val)] = t.ap()
    nc.all_engine_barrier()

    # ---- I/O -------------------------------------------------------------
    oht = nc.dram_tensor("oht", [P, NTOK], BF16, kind="ExternalInput").ap()
    embp = nc.dram_tensor("embp", [P, E], BF16, kind="ExternalInput").ap()
    pose = nc.dram_tensor("pose", [T, E], F32, kind="ExternalInput").ap()
    maskt = nc.dram_tensor("maskt", [P, P], BF16, kind="ExternalInput").ap()
    wq = nc.dram_tensor("wq", [L, E, E], BF16, kind="ExternalInput").ap()
    wk = nc.dram_tensor("wk", [L, E, E], BF16, kind="ExternalInput").ap()
    wv = nc.dram_tensor("wv", [L, E, E], BF16, kind="ExternalInput").ap()
    wo = nc.dram_tensor("wo", [L, E, E], BF16, kind="ExternalInput").ap()
    w1 = nc.dram_tensor("w1", [L, E, DFF], BF16, kind="ExternalInput").ap()
    w2 = nc.dram_tensor("w2", [L, DFF, E], BF16, kind="ExternalInput").ap()
    wl = nc.dram_tensor("wl", [E, V], BF16, kind="ExternalInput").ap()
    bqf = nc.dram_tensor("bqf", [L, P, EC], F32, kind="ExternalInput").ap()
    bkf = nc.dram_tensor("bkf", [L, P, EC], F32, kind="ExternalInput").ap()
    c1f = nc.dram_tensor("c1f", [L, P, FC], F32, kind="ExternalInput").ap()
    btm = nc.dram_tensor("btm", [L, 3, P, E], F32, kind="ExternalInput").ap()
    blr = nc.dram_tensor("blr", [P, V], F32, kind="ExternalInput").ap()
    out = nc.dram_tensor("out", [NTOK, V], F32, kind="ExternalOutput").ap()

    with tile.TileContext(nc) as tc, ExitStack() as es:
            ep = es.enter_context
            const = ep(tc.tile_pool(name="const", bufs=1))
            xres = ep(tc.tile_pool(name="xres", bufs=1))
            wa = ep(tc.tile_pool(name="wa", bufs=2))
            wf = ep(tc.tile_pool(name="wf", bufs=2))
            bias = ep(tc.tile_pool(name="bias", bufs=2))
            grp = ep(tc.tile_pool(name="grp", bufs=2))
            grp1 = ep(tc.tile_pool(name="grp1", bufs=1))
            vt = ep(tc.tile_pool(name="vt", bufs=6))
            tk = ep(tc.tile_pool(name="tk", bufs=4))
            ptp = ep(tc.tile_pool(name="ptp", bufs=6))
            otp = ep(tc.tile_pool(name="otp", bufs=2))
            stat = ep(tc.tile_pool(name="stat", bufs=8))
            psmm = ep(tc.tile_pool(name="psmm", bufs=2, space="PSUM"))
            pss = ep(tc.tile_pool(name="pss", bufs=2, space="PSUM"))
            psav = ep(tc.tile_pool(name="psav", bufs=2, space="PSUM"))
            pstr = ep(tc.tile_pool(name="pstr", bufs=2, space="PSUM"))
            # ---- constants ----
            id_bf = const.tile([P, P], BF16, tag="id_bf")
            make_identity(nc, id_bf)
            mask_sb = const.tile([P, P], BF16, tag="mask")
            nc.sync.dma_start(mask_sb[:], maskt[:])
            ones_col = const.tile([P, 1], BF16, tag="ones")
            nc.vector.memset(ones_col[:], 1.0)
            emb_sb = const.tile([P, E], BF16, tag="emb")
            nc.sync.dma_start(emb_sb[:], embp[:])
            pose_sb = const.tile([P, 2, E], F32, tag="pose")
            nc.sync.dma_start(pose_sb[:, 0, :], pose[0:P, :])
            nc.sync.dma_start(pose_sb[:, 1, :], pose[P : 2 * P, :])
            wl_sb = const.tile([P, EC, V], BF16, tag="wl")
            nc.sync.dma_start(wl_sb[:], wl.rearrange("(kc p) n -> p kc n", p=P))
            blr_sb = const.tile([P, V], F32, tag="blr")
            nc.sync.dma_start(blr_sb[:], blr[:])
            oht_sb = const.tile([P, NTOK], BF16, tag="oht")
            nc.sync.dma_start(oht_sb[:], oht[:])

            x_tm = [xres.tile([P, E], F32, tag=f"x{t}", name=f"x{t}") for t in range(NT)]

            NU = L * NG                  # pipelined (layer, group) units
            weights = {}                 # l -> weight/bias tiles
            st = {}                      # u -> inter-stage tiles

            def load_weights(l):
                w = {}
                w["wq"] = wa.tile([P, EC, E], BF16, tag="wq", name="wq_sb")
                nc.sync.dma_start(w["wq"][:], wq[l].rearrange("(kc p) n -> p kc n", p=P))
                w["wk"] = wa.tile([P, EC, E], BF16, tag="wk", name="wk_sb")
                nc.sync.dma_start(w["wk"][:], wk[l].rearrange("(kc p) n -> p kc n", p=P))
                w["wv"] = wa.tile([P, EC, E], BF16, tag="wv", name="wv_sb")
                nc.sync.dma_start(w["wv"][:], wv[l].rearrange("(kc p) n -> p kc n", p=P))
                w["wo"] = wa.tile([P, EC, E], BF16, tag="wo", name="wo_sb")
                nc.sync.dma_start(w["wo"][:], wo[l].rearrange("(kc p) n -> p kc n", p=P))
                w["w1"] = wf.tile([P, EC, DFF], BF16, tag="w1", name="w1_sb")
                nc.sync.dma_start(w["w1"][:], w1[l].rearrange("(kc p) n -> p kc n", p=P))
                w["w2"] = wf.tile([P, FC, E], BF16, tag="w2", name="w2_sb")
                nc.sync.dma_start(w["w2"][:], w2[l].rearrange("(kc p) n -> p kc n", p=P))
                if not zero_bias:
                    w["bq"] = bias.tile([P, EC], F32, tag="bq", name="bq_sb")
                    nc.sync.dma_start(w["bq"][:], bqf[l])
                    w["bk"] = bias.tile([P, EC], F32, tag="bk", name="bk_sb")
                    nc.sync.dma_start(w["bk"][:], bkf[l])
                    w["c1"] = bias.tile([P, FC], F32, tag="c1", name="c1_sb")
                    nc.sync.dma_start(w["c1"][:], c1f[l])
                    w["btm"] = bias.tile([P, 3, E], F32, tag="btm", name="btm_sb")
                    nc.sync.dma_start(w["btm"][:], btm[l].rearrange("t p n -> p t n"))
                weights[l] = w

            def stageA(u):
                """LN1 + QKV projections + attention -> o_tm tiles."""
                l, g = u // NG, u % NG
                w = weights[l]
                tts = [g * TPG + i for i in range(TPG)]

                h_fm = grp.tile([P, EC, GROUP], BF16, tag="hfm", name="h_fm")
                mv_g, rs_g = _ln_stats_group(nc, stat, [x_tm[tt][:] for tt in tts])
                for i, tt in enumerate(tts):
                    xh = tk.tile([P, E], BF16, tag="xh1", name="xh")
                    LN_ENG(nc).tensor_scalar(xh[:], x_tm[tt][:],
                                            mv_g[:, i, 0:1], rs_g[:, i : i + 1],
                                            op0=OP.subtract, op1=OP.mult)
                    ptt = pstr.tile([P, EC, P], BF16, tag="tr", name="ptt")
                    for kc in range(EC):
                        nc.tensor.transpose(ptt[:, kc, :],
                                            xh[:, kc * P : (kc + 1) * P], id_bf[:])
                    if i % 2 == 0:
                        nc.vector.tensor_copy(h_fm[:, :, i * P : (i + 1) * P], ptt[:])
                    else:
                        nc.scalar.copy(h_fm[:, :, i * P : (i + 1) * P], ptt[:])

                q_fm = grp.tile([P, EC, GROUP], BF16, tag="qfm", name="q_fm")
                k_fm = grp.tile([P, EC, GROUP], BF16, tag="kfm", name="k_fm")
                for dst, wsb, bname in ((q_fm, w["wq"], "bq"), (k_fm, w["wk"], "bk")):
                    for m in range(EC):
                        pq = psmm.tile([P, GROUP], F32, tag="mm", name="pq")
                        for kc in range(EC):
                            nc.tensor.matmul(pq[:], wsb[:, kc, m * P : (m + 1) * P],
                                             h_fm[:, kc, :],
                                             start=(kc == 0), stop=(kc == EC - 1))
                        if zero_bias:
                            nc.scalar.copy(dst[:, m, :], pq[:])
                        else:
                            nc.scalar.activation(dst[:, m, :], pq[:], AF.Identity,
                                                 bias=w[bname][:, m : m + 1], scale=1.0)

                v_tiles = []
                for i, tt in enumerate(tts):
                    pv = psmm.tile([P, GROUP], F32, tag="mm", name="pv")
                    for kc in range(EC):
                        nc.tensor.matmul(pv[:, :E], h_fm[:, kc, i * P : (i + 1) * P],
                                         w["wv"][:, kc, :],
                                         start=(kc == 0), stop=(kc == EC - 1))
                    vt_i = vt.tile([P, E], BF16, tag="vtm", name="vt_i")
                    if zero_bias:
                        nc.vector.tensor_copy(vt_i[:], pv[:, :E])
                    else:
                        nc.vector.tensor_tensor(vt_i[:], pv[:, :E], w["btm"][:, 0, :], OP.add)
                    v_tiles.append(vt_i)

                o_tm = [otp.tile([P, E], BF16, tag=f"otm{i}", name=f"otm{i}")
                        for i in range(TPG)]
                for lb in range(BPG):
                    v0 = v_tiles[2 * lb]
                    v1 = v_tiles[2 * lb + 1]
                    q0 = lb * T
                    for j in range(EC):          # head pair (2j, 2j+1)
                        pav = psav.tile([P, 260], F32, tag="av", name="pav")
                        for hh in range(2):
                            h = 2 * j + hh
                            ro = (h % 2) * 64
                            mc = h // 2
                            q_ap = q_fm[ro : ro + 64, mc, q0 : q0 + T]
                            k_ap = k_fm[ro : ro + 64, mc, q0 : q0 + T]

                            # scoresT [k, q]: cols 0:256 = ktile0 x q(0:256),
                            # cols 256:384 = ktile1 x q(128:256)
                            ps = pss.tile([P, E], F32, tag="s", name="ps")
                            nc.tensor.matmul(ps[:, 0:T], k_ap[:, 0:P], q_ap[:],
                                             start=True, stop=False)
                            nc.tensor.matmul(ps[:, 0:P], id_bf[:], mask_sb[:],
                                             start=False, stop=True)
                            nc.tensor.matmul(ps[:, T:E], k_ap[:, P:T], q_ap[:, P:T],
                                             start=True, stop=False)
                            nc.tensor.matmul(ps[:, T:E], id_bf[:], mask_sb[:],
                                             start=False, stop=True)
                            pt_sb = ptp.tile([P, E], BF16, tag="pt", name="pt_sb")
                            nc.scalar.activation(pt_sb[:], ps[:], AF.Exp,
                                                 bias=0.0, scale=1.0)

                            # AV token-major + ones-column sums
                            vsl = slice(h * 64, (h + 1) * 64)
                            c = hh * 64
                            sc = 256 + 2 * hh
                            nc.tensor.matmul(pav[:, c : c + 64], pt_sb[:, 0:P],
                                             v0[:, vsl], start=True, stop=True)
                            nc.tensor.matmul(pav[:, sc : sc + 1], pt_sb[:, 0:P],
                                             ones_col[:], start=True, stop=True)
                            nc.tensor.matmul(pav[:, 128 + c : 128 + c + 64],
                                             pt_sb[:, P:T], v0[:, vsl],
                                             start=True, stop=False)
                            nc.tensor.matmul(pav[:, 128 + c : 128 + c + 64],
                                             pt_sb[:, T:E], v1[:, vsl],
                                             start=False, stop=True)
                            nc.tensor.matmul(pav[:, sc + 1 : sc + 2],
                                             pt_sb[:, P:T], ones_col[:],
                                             start=True, stop=False)
                            nc.tensor.matmul(pav[:, sc + 1 : sc + 2],
                                             pt_sb[:, T:E], ones_col[:],
                                             start=False, stop=True)

                        # normalize on copy-out: cols 256:260 hold
                        # [s_h0_q0, s_h0_q1, s_h1_q0, s_h1_q1]
                        rs4 = stat.tile([P, 4], F32, tag="rs4", name="rs4")
                        nc.vector.reciprocal(rs4[:], pav[:, 256:260])
                        # (recip col, pav col, q-tile within pair)
                        for idx, (ri, pc, qi) in enumerate(
                                ((0, 0, 0), (1, 128, 1), (2, 64, 0), (3, 192, 1))):
                            htarget = 2 * j + (0 if idx < 2 else 1)
                            dst = o_tm[2 * lb + qi][:, htarget * 64 : htarget * 64 + 64]
                            if idx % 2 == 0:
                                nc.vector.tensor_scalar_mul(
                                    dst, pav[:, pc : pc + 64], rs4[:, ri : ri + 1])
                            else:
                                nc.scalar.activation(
                                    dst, pav[:, pc : pc + 64], AF.Copy,
                                    bias=0.0, scale=rs4[:, ri : ri + 1])
                st[u] = {"o_tm": o_tm}

            def stageB1(u):
                """o transpose + Wo + residual + LN2 stats/apply -> xh2 tiles."""
                l, g = u // NG, u % NG
                w = weights[l]
                tts = [g * TPG + i for i in range(TPG)]
                o_tm = st[u]["o_tm"]

                o_fm = grp.tile([P, EC, GROUP], BF16, tag="ofm", name="o_fm")
                for i in range(TPG):
                    pto = pstr.tile([P, EC, P], BF16, tag="tr", name="pto")
                    for kc in range(EC):
                        nc.tensor.transpose(pto[:, kc, :],
                                            o_tm[i][:, kc * P : (kc + 1) * P], id_bf[:])
                    if i % 2 == 0:
                        nc.scalar.copy(o_fm[:, :, i * P : (i + 1) * P], pto[:])
                    else:
                        nc.vector.tensor_copy(o_fm[:, :, i * P : (i + 1) * P], pto[:])

                for i, tt in enumerate(tts):
                    pao = psmm.tile([P, GROUP], F32, tag="mm", name="pao")
                    for kc in range(EC):
                        nc.tensor.matmul(pao[:, :E], o_fm[:, kc, i * P : (i + 1) * P],
                                         w["wo"][:, kc, :],
                                         start=(kc == 0), stop=(kc == EC - 1))
                    if zero_bias:
                        nc.vector.tensor_tensor(x_tm[tt][:], pao[:, :E], x_tm[tt][:], OP.add)
                    else:
                        t1 = tk.tile([P, E], F32, tag="t1", name="t1")
                        nc.vector.tensor_tensor(t1[:], pao[:, :E], x_tm[tt][:], OP.add)
                        nc.gpsimd.tensor_tensor(x_tm[tt][:], t1[:], w["btm"][:, 1, :], OP.add)

                mv_g2, rs_g2 = _ln_stats_group(nc, stat, [x_tm[tt][:] for tt in tts])
                xh2s = []
                for i, tt in enumerate(tts):
                    xh2 = tk.tile([P, E], BF16, tag="xh2", name="xh2")
                    LN_ENG(nc).tensor_scalar(xh2[:], x_tm[tt][:],
                                            mv_g2[:, i, 0:1], rs_g2[:, i : i + 1],
                                            op0=OP.subtract, op1=OP.mult)
                    xh2s.append(xh2)
                st[u]["xh2"] = xh2s

            def stageB2(u):
                """LN2 transposes -> h2_fm."""
                h2_fm = grp.tile([P, EC, GROUP], BF16, tag="h2fm", name="h2_fm")
                for i, xh2 in enumerate(st[u]["xh2"]):
                    ptt2 = pstr.tile([P, EC, P], BF16, tag="tr", name="ptt2")
                    for kc in range(EC):
                        nc.tensor.transpose(ptt2[:, kc, :],
                                            xh2[:, kc * P : (kc + 1) * P], id_bf[:])
                    if i % 2 == 0:
                        nc.scalar.copy(h2_fm[:, :, i * P : (i + 1) * P], ptt2[:])
                    else:
                        nc.vector.tensor_copy(h2_fm[:, :, i * P : (i + 1) * P], ptt2[:])
                st[u]["h2_fm"] = h2_fm

            def stageC(u):
                """FFN: W1+relu, W2+residual."""
                l, g = u // NG, u % NG
                w = weights[l]
                tts = [g * TPG + i for i in range(TPG)]
                h2_fm = st[u]["h2_fm"]

                hf = grp1.tile([P, FC, GROUP], BF16, tag="hf", name="hf")
                for m in range(FC):
                    pf = psmm.tile([P, GROUP], F32, tag="mm", name="pf")
                    for kc in range(EC):
                        nc.tensor.matmul(pf[:], w["w1"][:, kc, m * P : (m + 1) * P],
                                         h2_fm[:, kc, :],
                                         start=(kc == 0), stop=(kc == EC - 1))
                    if zero_bias:
                        if m % 3 == 2:
                            nc.vector.tensor_scalar_max(hf[:, m, :], pf[:], 0.0)
                        else:
                            nc.scalar.activation(hf[:, m, :], pf[:], AF.Relu,
                                                 bias=0.0, scale=1.0)
                    else:
                        if m % 3 == 2:
                            nc.vector.tensor_scalar(hf[:, m, :], pf[:],
                                                    w["c1"][:, m : m + 1], 0.0,
                                                    op0=OP.add, op1=OP.max)
                        else:
                            nc.scalar.activation(hf[:, m, :], pf[:], AF.Relu,
                                                 bias=w["c1"][:, m : m + 1], scale=1.0)

                for i, tt in enumerate(tts):
                    pw2 = psmm.tile([P, GROUP], F32, tag="mm", name="pw2")
                    for kc in range(FC):
                        nc.tensor.matmul(pw2[:, :E], hf[:, kc, i * P : (i + 1) * P],
                                         w["w2"][:, kc, :],
                                         start=(kc == 0), stop=(kc == FC - 1))
                    if zero_bias:
                        nc.vector.tensor_tensor(x_tm[tt][:], pw2[:, :E], x_tm[tt][:], OP.add)
                    else:
                        t2 = tk.tile([P, E], F32, tag="t1", name="t2")
                        nc.vector.tensor_tensor(t2[:], pw2[:, :E], x_tm[tt][:], OP.add)
                        nc.gpsimd.tensor_tensor(x_tm[tt][:], t2[:], w["btm"][:, 2, :], OP.add)
                del st[u]

            for _rep in range(repeat):
                # ---- x0 = onehot @ emb + pos ----
                for tt in range(NT):
                    xt = x_tm[tt]
                    pe = psmm.tile([P, GROUP], F32, tag="mm", name="pe")
                    nc.tensor.matmul(pe[:, :E], oht_sb[:, tt * P : (tt + 1) * P],
                                     emb_sb[:], start=True, stop=True)
                    nc.vector.tensor_tensor(xt[:], pe[:, :E], pose_sb[:, tt % 2, :], OP.add)

                # ---- pipelined layers: A(u) | B1(u-1) | B2(u-2), C(u-2) ----
                load_weights(0)
                for u in range(NU):
                    l, g = u // NG, u % NG
                    if g == 2 and l + 1 < L:
                        load_weights(l + 1)
                    stageA(u)
                    if u >= 1:
                        stageB1(u - 1)
                    if u >= 2:
                        stageB2(u - 2)
                        stageC(u - 2)
                stageB1(NU - 1)
                stageB2(NU - 2)
                stageC(NU - 2)
                stageB2(NU - 1)
                stageC(NU - 1)

                # ---- final logits ----
                for tt in range(NT):
                    xb = tk.tile([P, E], BF16, tag="xhat")
                    nc.any.tensor_copy(out=xb[:], in_=x_tm[tt][:])
                    ptl = pstr.tile([P, EC, P], BF16, tag="tr")
                    for kc in range(EC):
                        nc.tensor.transpose(ptl[:, kc, :],
                                            xb[:, kc * P : (kc + 1) * P], id_bf[:])
                    xf = tk.tile([P, EC, P], BF16, tag="xf")
                    if tt % 2 == 0:
                        nc.vector.tensor_copy(xf[:], ptl[:])
                    else:
                        nc.scalar.copy(xf[:], ptl[:])
                    pl = psmm.tile([P, GROUP], F32, tag="mm")
                    for kc in range(EC):
                        nc.tensor.matmul(pl[:, :V], xf[:, kc, :], wl_sb[:, kc, :],
                                         start=(kc == 0), stop=(kc == EC - 1))
                    lg = tk.tile([P, V], F32, tag="lg")
                    if zero_bias:
                        if tt % 2 == 0:
                            nc.scalar.copy(lg[:], pl[:, :V])
                        else:
                            nc.vector.tensor_copy(lg[:], pl[:, :V])
                    else:
                        nc.vector.tensor_tensor(lg[:], pl[:, :V], blr_sb[:], OP.add)
                    nc.sync.dma_start(out[tt * P : (tt + 1) * P, :], lg[:])

    nc.compile()
    return nc


def _prep_host(inputs):
    f32 = np.float32
    bf16 = ml_dtypes.bfloat16
    tokens = np.asarray(inputs["tokens"]).astype(np.int64)
    emb = np.asarray(inputs["emb"], dtype=f32)
    pos_enc = np.asarray(inputs["pos_enc"], dtype=f32)
    Wq = np.asarray(inputs["Wq"], dtype=f32)
    Wk = np.asarray(inputs["Wk"], dtype=f32)
    Wv = np.asarray(inputs["Wv"], dtype=f32)
    Wo = np.asarray(inputs["Wo"], dtype=f32)
    W1 = np.asarray(inputs["W1"], dtype=f32)
    W2 = np.asarray(inputs["W2"], dtype=f32)
    Wl = np.asarray(inputs["Wl"], dtype=f32)
    bq = np.asarray(inputs["bq"], dtype=f32)
    bk = np.asarray(inputs["bk"], dtype=f32)
    bv = np.asarray(inputs["bv"], dtype=f32)
    bo = np.asarray(inputs["bo"], dtype=f32)
    c1 = np.asarray(inputs["c1"], dtype=f32)
    c2 = np.asarray(inputs["c2"], dtype=f32)
    bl = np.asarray(inputs["bl"], dtype=f32)
    g1 = np.asarray(inputs["ln1_g"], dtype=f32)
    b1 = np.asarray(inputs["ln1_b"], dtype=f32)
    g2 = np.asarray(inputs["ln2_g"], dtype=f32)
    b2 = np.asarray(inputs["ln2_b"], dtype=f32)

    scale = D ** -0.5
    wq_f = np.empty((L, E, E), f32)
    wk_f = np.empty((L, E, E), f32)
    wv_f = np.empty((L, E, E), f32)
    w1_f = np.empty((L, E, DFF), f32)
    bq_f = np.empty((L, E), f32)
    bk_f = np.empty((L, E), f32)
    bv_f = np.empty((L, E), f32)
    c1_f = np.empty((L, DFF), f32)
    for l in range(L):
        wq_f[l] = g1[l][:, None] * Wq[l] * scale
        bq_f[l] = (b1[l] @ Wq[l] + bq[l]) * scale
        wk_f[l] = g1[l][:, None] * Wk[l]
        bk_f[l] = b1[l] @ Wk[l] + bk[l]
        wv_f[l] = g1[l][:, None] * Wv[l]
        bv_f[l] = b1[l] @ Wv[l] + bv[l]
        w1_f[l] = g2[l][:, None] * W1[l]
        c1_f[l] = b2[l] @ W1[l] + c1[l]

    # maskt[k, q] = 0 if k <= q else NEG  (transposed causal mask)
    maskt = np.where(np.tril(np.ones((P, P), bool)).T, 0.0, NEG).astype(bf16)

    common = {
        "embp": np.zeros((P, E), bf16),
        "pose": pos_enc,
        "maskt": maskt,
        "wq": wq_f.astype(bf16),
        "wk": wk_f.astype(bf16),
        "wv": wv_f.astype(bf16),
        "wo": Wo.astype(bf16),
        "w1": w1_f.astype(bf16),
        "w2": W2.astype(bf16),
        "wl": Wl.astype(bf16),
        "bqf": np.ascontiguousarray(bq_f.reshape(L, EC, P).transpose(0, 2, 1)),
        "bkf": np.ascontiguousarray(bk_f.reshape(L, EC, P).transpose(0, 2, 1)),
        "c1f": np.ascontiguousarray(c1_f.reshape(L, FC, P).transpose(0, 2, 1)),
        "btm": np.ascontiguousarray(
            np.broadcast_to(
                np.stack([bv_f, bo, c2], axis=1)[:, :, None, :], (L, 3, P, E)
            )
        ).astype(f32),
        "blr": np.broadcast_to(bl[None, :], (P, V)).astype(f32),
    }
    common["embp"][:V, :] = emb.astype(bf16)

    in_maps = []
    for c in range(N_CORES):
        tok_c = tokens[c * B_LOC : (c + 1) * B_LOC].reshape(-1)
        oht = np.zeros((P, NTOK), bf16)
        oht[tok_c, np.arange(NTOK)] = 1
        m = dict(common)
        m["oht"] = oht
        in_maps.append(m)
    return in_maps


def _biases_all_zero(inputs):
    zs = [inputs[k] for k in ("bq", "bk", "bv", "bo", "c1", "c2", "bl",
                              "ln1_b", "ln2_b")]
    return all(not np.any(np.asarray(z)) for z in zs)


def kernel(**inputs) -> np.ndarray:
    global _PROG
    zb = _biases_all_zero(inputs)
    if _PROG is None or _PROG[1] != zb:
        _PROG = (build_program(zero_bias=zb), zb)
    nc = _PROG[0]
    in_maps = _prep_host(inputs)
    res = run_bass_kernel_spmd(nc, in_maps, list(range(N_CORES)))
    outs = [res.results[c]["out"].reshape(B_LOC, T, V) for c in range(N_CORES)]
    return np.concatenate(outs, axis=0).astype(np.float32)


# revision 13
# speedup vs baseline: 2.0651x; 1.0720x over previous
"""Trainium2 Bass kernel for a 6-layer causal decoder transformer (v2).

Model: B=128, T=256, E=384, H=6, D=64, DFF=1536, L=6, V=65 (f32 reference).
Sharding: pure data-parallel over batch across 8 NeuronCores (16 batches
per core), parameters replicated, no collectives.

v2 changes vs v1 (engine-balance redesign, from CoreSim cost analysis):
  - Scores computed TRANSPOSED on PE (scoresT[k,q] = k^T q) so the exp'd
    probabilities are already in the layout the AV matmul needs as lhsT —
    the 3 per-(batch,head) P-transposes and their PSUM copy-outs are gone.
  - One [128,384] scoresT PSUM tile per (batch,head): ktile0 x q(0:256) in
    cols 0:256, ktile1 x q(128:256) in cols 256:384; causal mask added by
    PE (identity @ maskT accumulate) on the two diagonal blocks; ONE exp
    activation per head (no accum_out — softmax denominators come from
    N=1 ones-column matmuls accumulated into the AV PSUM tile).
  - AV output token-major [q, d] (N=64 matmuls), normalized during the
    PSUM->SBUF copy by per-partition reciprocal scaling; o then transposed
    to feature-major for the Wo matmul (3 PE transposes + ONE batched copy
    per token tile).
  - LayerNorm x-hat apply runs on the otherwise-idle GpSimd engine.
  - Transpose copy-outs batched ([P,3,128] PSUM -> one strided copy).
  - PSUM->SBUF copy-outs hand-balanced across ScalarE/VectorE.
"""

import sys
from contextlib import ExitStack

sys.path.insert(0, "/opt/trn_rl_repo")

import numpy as np
import ml_dtypes

import concourse.bass as bass
import concourse.bacc as bacc
import concourse.mybir as mybir
import concourse.tile as tile
from concourse.masks import make_identity
from concourse.bass_utils import run_bass_kernel_spmd

F32 = mybir.dt.float32
BF16 = mybir.dt.bfloat16
AF = mybir.ActivationFunctionType
OP = mybir.AluOpType

P = 128
E, DFF, H, D, T, L, V = 384, 1536, 6, 64, 256, 6, 65
B = 128
N_CORES = 8
B_LOC = B // N_CORES          # 16 batches per core
NTOK = B_LOC * T              # 4096 tokens per core
NT = NTOK // P                # 32 token tiles
GROUP = 512                   # tokens per group (2 full batches)
NG = NTOK // GROUP            # 8 groups
TPG = GROUP // P              # 4 token tiles per group
BPG = GROUP // T              # 2 batches per group
EC = E // P                   # 3 feature chunks
FC = DFF // P                 # 12 dff chunks
NEG = -1.0e9

_PROG = None  # (nc, zero_bias)
LN_ENG = lambda nc: nc.vector  # engine for LN x-hat apply (A/B testable)


def _ln_stats_group(nc, stat, x_list, eps=1e-5):
    """bn_stats per tile + batched Newton rsqrt. Returns (mv_g, rs_g):
    mv_g[:, i, 0:1] = mean of tile i; rs_g[:, i:i+1] = rsqrt(var_i + eps)."""
    n = len(x_list)
    mv_g = stat.tile([P, n, 2], F32, tag="mvg")
    for i, xin in enumerate(x_list):
        st6 = stat.tile([P, 6], F32, tag="bn6")
        nc.vector.bn_stats(out=st6[:], in_=xin)
        nc.vector.bn_aggr(out=mv_g[:, i, :], in_=st6[:])
    var = stat.tile([P, n], F32, tag="vare")
    nc.vector.tensor_scalar_add(var[:], mv_g[:, :, 1], eps)
    u = stat.tile([P, n], F32, tag="ue")
    nc.vector.reciprocal(u[:], var[:])
    lin = stat.tile([P, n], F32, tag="line")
    nc.vector.tensor_scalar(lin[:], var[:], 0.73, 0.32, op0=OP.mult, op1=OP.add)
    rs = stat.tile([P, n], F32, tag="rse")
    nc.vector.tensor_tensor(rs[:], u[:], lin[:], OP.mult)       # seed ~ rsqrt
    t1 = stat.tile([P, n], F32, tag="t1e")
    for _ in range(2):                                          # Newton x2
        nc.vector.tensor_tensor(t1[:], rs[:], rs[:], OP.mult)
        nc.vector.tensor_tensor(t1[:], t1[:], var[:], OP.mult)
        nc.vector.tensor_scalar(t1[:], t1[:], -0.5, 1.5, op0=OP.mult, op1=OP.add)
        nc.vector.tensor_tensor(rs[:], rs[:], t1[:], OP.mult)
    return mv_g, rs


def build_program(repeat=1, dma_t=False, zero_bias=True):
    nc = bacc.Bacc("TRN2", target_bir_lowering=False, debug=False,
                   num_devices=N_CORES)

    # register const APs needed for float biases on ScalarE activations
    for val in (1e-5,):
        t = nc.alloc_sbuf_tensor(f"const-f32-{val}", [P, 1], F32)
        nc.gpsimd.memset(t.ap(), val)
        nc.const_aps.aps[(F32, val)] = t.ap()
    nc.all_engine_barrier()

    # ---- I/O -------------------------------------------------------------
    oht = nc.dram_tensor("oht", [P, NTOK], BF16, kind="ExternalInput").ap()
    embp = nc.dram_tensor("embp", [P, E], BF16, kind="ExternalInput").ap()
    pose = nc.dram_tensor("pose", [T, E], F32, kind="ExternalInput").ap()
    maskt = nc.dram_tensor("maskt", [P, P], BF16, kind="ExternalInput").ap()
    wq = nc.dram_tensor("wq", [L, E, E], BF16, kind="ExternalInput").ap()
    wk = nc.dram_tensor("wk", [L, E, E], BF16, kind="ExternalInput").ap()
    wv = nc.dram_tensor("wv", [L, E, E], BF16, kind="ExternalInput").ap()
    wo = nc.dram_tensor("wo", [L, E, E], BF16, kind="ExternalInput").ap()
    w1 = nc.dram_tensor("w1", [L, E, DFF], BF16, kind="ExternalInput").ap()
    w2 = nc.dram_tensor("w2", [L, DFF, E], BF16, kind="ExternalInput").ap()
    wl = nc.dram_tensor("wl", [E, V], BF16, kind="ExternalInput").ap()
    bqf = nc.dram_tensor("bqf", [L, P, EC], F32, kind="ExternalInput").ap()
    bkf = nc.dram_tensor("bkf", [L, P, EC], F32, kind="ExternalInput").ap()
    c1f = nc.dram_tensor("c1f", [L, P, FC], F32, kind="ExternalInput").ap()
    btm = nc.dram_tensor("btm", [L, 3, P, E], F32, kind="ExternalInput").ap()
    blr = nc.dram_tensor("blr", [P, V], F32, kind="ExternalInput").ap()
    out = nc.dram_tensor("out", [NTOK, V], F32, kind="ExternalOutput").ap()

    with tile.TileContext(nc) as tc, ExitStack() as es:
            ep = es.enter_context
            const = ep(tc.tile_pool(name="const", bufs=1))
            xres = ep(tc.tile_pool(name="xres", bufs=1))
            wa = ep(tc.tile_pool(name="wa", bufs=2))
            wf = ep(tc.tile_pool(name="wf", bufs=2))
            bias = ep(tc.tile_pool(name="bias", bufs=2))
            grp = ep(tc.tile_pool(name="grp", bufs=2))
            grp1 = ep(tc.tile_pool(name="grp1", bufs=1))
            vt = ep(tc.tile_pool(name="vt", bufs=6))
            tk = ep(tc.tile_pool(name="tk", bufs=4))
            ptp = ep(tc.tile_pool(name="ptp", bufs=6))
            otp = ep(tc.tile_pool(name="otp", bufs=3))
            xh2p = ep(tc.tile_pool(name="xh2p", bufs=8))
            stat = ep(tc.tile_pool(name="stat", bufs=8))
            psmm = ep(tc.tile_pool(name="psmm", bufs=2, space="PSUM"))
            pss = ep(tc.tile_pool(name="pss", bufs=2, space="PSUM"))
            psav = ep(tc.tile_pool(name="psav", bufs=2, space="PSUM"))
            pstr = ep(tc.tile_pool(name="pstr", bufs=2, space="PSUM"))
            # ---- constants ----
            id_bf = const.tile([P, P], BF16, tag="id_bf")
            make_identity(nc, id_bf)
            mask_sb = const.tile([P, P], BF16, tag="mask")
            nc.sync.dma_start(mask_sb[:], maskt[:])
            ones_col = const.tile([P, 1], BF16, tag="ones")
            nc.vector.memset(ones_col[:], 1.0)
            emb_sb = const.tile([P, E], BF16, tag="emb")
            nc.sync.dma_start(emb_sb[:], embp[:])
            pose_sb = const.tile([P, 2, E], F32, tag="pose")
            nc.sync.dma_start(pose_sb[:, 0, :], pose[0:P, :])
            nc.sync.dma_start(pose_sb[:, 1, :], pose[P : 2 * P, :])
            wl_sb = const.tile([P, EC, V], BF16, tag="wl")
            nc.sync.dma_start(wl_sb[:], wl.rearrange("(kc p) n -> p kc n", p=P))
            blr_sb = const.tile([P, V], F32, tag="blr")
            nc.sync.dma_start(blr_sb[:], blr[:])
            oht_sb = const.tile([P, NTOK], BF16, tag="oht")
            nc.sync.dma_start(oht_sb[:], oht[:])

            x_tm = [xres.tile([P, E], F32, tag=f"x{t}", name=f"x{t}") for t in range(NT)]

            NU = L * NG                  # pipelined (layer, group) units
            weights = {}                 # l -> weight/bias tiles
            st = {}                      # u -> inter-stage tiles

            def load_weights(l):
                w = {}
                w["wq"] = wa.tile([P, EC, E], BF16, tag="wq", name="wq_sb")
                nc.sync.dma_start(w["wq"][:], wq[l].rearrange("(kc p) n -> p kc n", p=P))
                w["wk"] = wa.tile([P, EC, E], BF16, tag="wk", name="wk_sb")
                nc.sync.dma_start(w["wk"][:], wk[l].rearrange("(kc p) n -> p kc n", p=P))
                w["wv"] = wa.tile([P, EC, E], BF16, tag="wv", name="wv_sb")
                nc.sync.dma_start(w["wv"][:], wv[l].rearrange("(kc p) n -> p kc n", p=P))
                w["wo"] = wa.tile([P, EC, E], BF16, tag="wo", name="wo_sb")
                nc.sync.dma_start(w["wo"][:], wo[l].rearrange("(kc p) n -> p kc n", p=P))
                w["w1"] = wf.tile([P, EC, DFF], BF16, tag="w1", name="w1_sb")
                nc.sync.dma_start(w["w1"][:], w1[l].rearrange("(kc p) n -> p kc n", p=P))
                w["w2"] = wf.tile([P, FC, E], BF16, tag="w2", name="w2_sb")
                nc.sync.dma_start(w["w2"][:], w2[l].rearrange("(kc p) n -> p kc n", p=P))
                if not zero_bias:
                    w["bq"] = bias.tile([P, EC], F32, tag="bq", name="bq_sb")
                    nc.sync.dma_start(w["bq"][:], bqf[l])
                    w["bk"] = bias.tile([P, EC], F32, tag="bk", name="bk_sb")
                    nc.sync.dma_start(w["bk"][:], bkf[l])
                    w["c1"] = bias.tile([P, FC], F32, tag="c1", name="c1_sb")
                    nc.sync.dma_start(w["c1"][:], c1f[l])
                    w["btm"] = bias.tile([P, 3, E], F32, tag="btm", name="btm_sb")
                    nc.sync.dma_start(w["btm"][:], btm[l].rearrange("t p n -> p t n"))
                weights[l] = w

            def stageA(u):
                """LN1 + QKV projections + attention -> o_tm tiles."""
                l, g = u // NG, u % NG
                w = weights[l]
                tts = [g * TPG + i for i in range(TPG)]

                h_fm = grp.tile([P, EC, GROUP], BF16, tag="hfm", name="h_fm")
                mv_g, rs_g = _ln_stats_group(nc, stat, [x_tm[tt][:] for tt in tts])
                for i, tt in enumerate(tts):
                    xh = tk.tile([P, E], BF16, tag="xh1", name="xh")
                    LN_ENG(nc).tensor_scalar(xh[:], x_tm[tt][:],
                                            mv_g[:, i, 0:1], rs_g[:, i : i + 1],
                                            op0=OP.subtract, op1=OP.mult)
                    ptt = pstr.tile([P, EC, P], BF16, tag="tr", name="ptt")
                    for kc in range(EC):
                        nc.tensor.transpose(ptt[:, kc, :],
                                            xh[:, kc * P : (kc + 1) * P], id_bf[:])
                    if i % 2 == 0:
                        nc.vector.tensor_copy(h_fm[:, :, i * P : (i + 1) * P], ptt[:])
                    else:
                        nc.scalar.copy(h_fm[:, :, i * P : (i + 1) * P], ptt[:])

                q_fm = grp.tile([P, EC, GROUP], BF16, tag="qfm", name="q_fm")
                k_fm = grp.tile([P, EC, GROUP], BF16, tag="kfm", name="k_fm")
                for dst, wsb, bname in ((q_fm, w["wq"], "bq"), (k_fm, w["wk"], "bk")):
                    for m in range(EC):
                        pq = psmm.tile([P, GROUP], F32, tag="mm", name="pq")
                        for kc in range(EC):
                            nc.tensor.matmul(pq[:], wsb[:, kc, m * P : (m + 1) * P],
                                             h_fm[:, kc, :],
                                             start=(kc == 0), stop=(kc == EC - 1))
                        if zero_bias:
                            nc.scalar.copy(dst[:, m, :], pq[:])
                        else:
                            nc.scalar.activation(dst[:, m, :], pq[:], AF.Identity,
                                                 bias=w[bname][:, m : m + 1], scale=1.0)

                v_tiles = []
                for i, tt in enumerate(tts):
                    pv = psmm.tile([P, GROUP], F32, tag="mm", name="pv")
                    for kc in range(EC):
                        nc.tensor.matmul(pv[:, :E], h_fm[:, kc, i * P : (i + 1) * P],
                                         w["wv"][:, kc, :],
                                         start=(kc == 0), stop=(kc == EC - 1))
                    vt_i = vt.tile([P, E], BF16, tag="vtm", name="vt_i")
                    if zero_bias:
                        nc.vector.tensor_copy(vt_i[:], pv[:, :E])
                    else:
                        nc.vector.tensor_tensor(vt_i[:], pv[:, :E], w["btm"][:, 0, :], OP.add)
                    v_tiles.append(vt_i)

                o_tm = [otp.tile([P, E], BF16, tag=f"otm{i}", name=f"otm{i}")
                        for i in range(TPG)]
                for lb in range(BPG):
                    v0 = v_tiles[2 * lb]
                    v1 = v_tiles[2 * lb + 1]
                    q0 = lb * T
                    for j in range(EC):          # head pair (2j, 2j+1)
                        pav = psav.tile([P, 260], F32, tag="av", name="pav")
                        for hh in range(2):
                            h = 2 * j + hh
                            ro = (h % 2) * 64
                            mc = h // 2
                            q_ap = q_fm[ro : ro + 64, mc, q0 : q0 + T]
                            k_ap = k_fm[ro : ro + 64, mc, q0 : q0 + T]

                            # scoresT [k, q]: cols 0:256 = ktile0 x q(0:256),
                            # cols 256:384 = ktile1 x q(128:256)
                            ps = pss.tile([P, E], F32, tag="s", name="ps")
                            nc.tensor.matmul(ps[:, 0:T], k_ap[:, 0:P], q_ap[:],
                                             start=True, stop=False)
                            nc.tensor.matmul(ps[:, 0:P], id_bf[:], mask_sb[:],
                                             start=False, stop=True)
                            nc.tensor.matmul(ps[:, T:E], k_ap[:, P:T], q_ap[:, P:T],
                                             start=True, stop=False)
                            nc.tensor.matmul(ps[:, T:E], id_bf[:], mask_sb[:],
                                             start=False, stop=True)
                            pt_sb = ptp.tile([P, E], BF16, tag="pt", name="pt_sb")
                            nc.scalar.activation(pt_sb[:], ps[:], AF.Exp,
                                                 bias=0.0, scale=1.0)

                            # AV token-major + ones-column sums
                            vsl = slice(h * 64, (h + 1) * 64)
                            c = hh * 64
                            sc = 256 + 2 * hh
                            nc.tensor.matmul(pav[:, c : c + 64], pt_sb[:, 0:P],
                                             v0[:, vsl], start=True, stop=True)
                            nc.tensor.matmul(pav[:, sc : sc + 1], pt_sb[:, 0:P],
                                             ones_col[:], start=True, stop=True)
                            nc.tensor.matmul(pav[:, 128 + c : 128 + c + 64],
                                             pt_sb[:, P:T], v0[:, vsl],
                                             start=True, stop=False)
                            nc.tensor.matmul(pav[:, 128 + c : 128 + c + 64],
                                             pt_sb[:, T:E], v1[:, vsl],
                                             start=False, stop=True)
                            nc.tensor.matmul(pav[:, sc + 1 : sc + 2],
                                             pt_sb[:, P:T], ones_col[:],
                                             start=True, stop=False)
                            nc.tensor.matmul(pav[:, sc + 1 : sc + 2],
                                             pt_sb[:, T:E], ones_col[:],
                                             start=False, stop=True)

                        # normalize on copy-out: cols 256:260 hold
                        # [s_h0_q0, s_h0_q1, s_h1_q0, s_h1_q1]
                        rs4 = stat.tile([P, 4], F32, tag="rs4", name="rs4")
                        nc.vector.reciprocal(rs4[:], pav[:, 256:260])
                        # (recip col, pav col, q-tile within pair)
                        for idx, (ri, pc, qi) in enumerate(
                                ((0, 0, 0), (1, 128, 1), (2, 64, 0), (3, 192, 1))):
                            htarget = 2 * j + (0 if idx < 2 else 1)
                            dst = o_tm[2 * lb + qi][:, htarget * 64 : htarget * 64 + 64]
                            if idx % 2 == 0:
                                nc.vector.tensor_scalar_mul(
                                    dst, pav[:, pc : pc + 64], rs4[:, ri : ri + 1])
                            else:
                                nc.scalar.activation(
                                    dst, pav[:, pc : pc + 64], AF.Copy,
                                    bias=0.0, scale=rs4[:, ri : ri + 1])
                st[u] = {"o_tm": o_tm}

            def stageB1(u):
                """o transpose + Wo + residual + LN2 stats/apply -> xh2 tiles."""
                l, g = u // NG, u % NG
                w = weights[l]
                tts = [g * TPG + i for i in range(TPG)]
                o_tm = st[u]["o_tm"]

                o_fm = grp.tile([P, EC, GROUP], BF16, tag="ofm", name="o_fm")
                for i in range(TPG):
                    pto = pstr.tile([P, EC, P], BF16, tag="tr", name="pto")
                    for kc in range(EC):
                        nc.tensor.transpose(pto[:, kc, :],
                                            o_tm[i][:, kc * P : (kc + 1) * P], id_bf[:])
                    if i % 2 == 0:
                        nc.scalar.copy(o_fm[:, :, i * P : (i + 1) * P], pto[:])
                    else:
                        nc.vector.tensor_copy(o_fm[:, :, i * P : (i + 1) * P], pto[:])

                for i, tt in enumerate(tts):
                    pao = psmm.tile([P, GROUP], F32, tag="mm", name="pao")
                    for kc in range(EC):
                        nc.tensor.matmul(pao[:, :E], o_fm[:, kc, i * P : (i + 1) * P],
                                         w["wo"][:, kc, :],
                                         start=(kc == 0), stop=(kc == EC - 1))
                    if zero_bias:
                        nc.vector.tensor_tensor(x_tm[tt][:], pao[:, :E], x_tm[tt][:], OP.add)
                    else:
                        t1 = tk.tile([P, E], F32, tag="t1", name="t1")
                        nc.vector.tensor_tensor(t1[:], pao[:, :E], x_tm[tt][:], OP.add)
                        nc.gpsimd.tensor_tensor(x_tm[tt][:], t1[:], w["btm"][:, 1, :], OP.add)

                mv_g2, rs_g2 = _ln_stats_group(nc, stat, [x_tm[tt][:] for tt in tts])
                xh2s = []
                for i, tt in enumerate(tts):
                    xh2 = xh2p.tile([P, E], BF16, tag="xh2", name="xh2")
                    LN_ENG(nc).tensor_scalar(xh2[:], x_tm[tt][:],
                                            mv_g2[:, i, 0:1], rs_g2[:, i : i + 1],
                                            op0=OP.subtract, op1=OP.mult)
                    xh2s.append(xh2)
                st[u]["xh2"] = xh2s

            def stageB2(u):
                """LN2 transposes -> h2_fm."""
                h2_fm = grp.tile([P, EC, GROUP], BF16, tag="h2fm", name="h2_fm")
                for i, xh2 in enumerate(st[u]["xh2"]):
                    ptt2 = pstr.tile([P, EC, P], BF16, tag="tr", name="ptt2")
                    for kc in range(EC):
                        nc.tensor.transpose(ptt2[:, kc, :],
                                            xh2[:, kc * P : (kc + 1) * P], id_bf[:])
                    if i % 2 == 0:
                        nc.scalar.copy(h2_fm[:, :, i * P : (i + 1) * P], ptt2[:])
                    else:
                        nc.vector.tensor_copy(h2_fm[:, :, i * P : (i + 1) * P], ptt2[:])
                st[u]["h2_fm"] = h2_fm

            def stageC(u):
                """FFN: W1+relu, W2+residual."""
                l, g = u // NG, u % NG
                w = weights[l]
                tts = [g * TPG + i for i in range(TPG)]
                h2_fm = st[u]["h2_fm"]

                hf = grp1.tile([P, FC, GROUP], BF16, tag="hf", name="hf")
                for m in range(FC):
                    pf = psmm.tile([P, GROUP], F32, tag="mm", name="pf")
                    for kc in range(EC):
                        nc.tensor.matmul(pf[:], w["w1"][:, kc, m * P : (m + 1) * P],
                                         h2_fm[:, kc, :],
                                         start=(kc == 0), stop=(kc == EC - 1))
                    if zero_bias:
                        if m % 3 == 2:
                            nc.vector.tensor_scalar_max(hf[:, m, :], pf[:], 0.0)
                        else:
                            nc.scalar.activation(hf[:, m, :], pf[:], AF.Relu,
                                                 bias=0.0, scale=1.0)
                    else:
                        if m % 3 == 2:
                            nc.vector.tensor_scalar(hf[:, m, :], pf[:],
                                                    w["c1"][:, m : m + 1], 0.0,
                                                    op0=OP.add, op1=OP.max)
                        else:
                            nc.scalar.activation(hf[:, m, :], pf[:], AF.Relu,
                                                 bias=w["c1"][:, m : m + 1], scale=1.0)

                for i, tt in enumerate(tts):
                    pw2 = psmm.tile([P, GROUP], F32, tag="mm", name="pw2")
                    for kc in range(FC):
                        nc.tensor.matmul(pw2[:, :E], hf[:, kc, i * P : (i + 1) * P],
                                         w["w2"][:, kc, :],
                                         start=(kc == 0), stop=(kc == FC - 1))
                    if zero_bias:
                        nc.vector.tensor_tensor(x_tm[tt][:], pw2[:, :E], x_tm[tt][:], OP.add)
                    else:
                        t2 = tk.tile([P, E], F32, tag="t1", name="t2")
                        nc.vector.tensor_tensor(t2[:], pw2[:, :E], x_tm[tt][:], OP.add)
                        nc.gpsimd.tensor_tensor(x_tm[tt][:], t2[:], w["btm"][:, 2, :], OP.add)
                del st[u]

            for _rep in range(repeat):
                # ---- x0 = onehot @ emb + pos ----
                for tt in range(NT):
                    xt = x_tm[tt]
                    pe = psmm.tile([P, GROUP], F32, tag="mm", name="pe")
                    nc.tensor.matmul(pe[:, :E], oht_sb[:, tt * P : (tt + 1) * P],
                                     emb_sb[:], start=True, stop=True)
                    nc.vector.tensor_tensor(xt[:], pe[:, :E], pose_sb[:, tt % 2, :], OP.add)

                # ---- pipelined layers: A(u) | B1(u-1) | B2(u-2), C(u-2) ----
                load_weights(0)
                for u in range(NU):
                    l, g = u // NG, u % NG
                    if g == 2 and l + 1 < L:
                        load_weights(l + 1)
                    stageA(u)
                    if u >= 1:
                        stageB1(u - 1)
                    if u >= 2:
                        stageB2(u - 2)
                        stageC(u - 2)
                stageB1(NU - 1)
                stageB2(NU - 2)
                stageC(NU - 2)
                stageB2(NU - 1)
                stageC(NU - 1)

                # ---- final logits ----
                for tt in range(NT):
                    xb = tk.tile([P, E], BF16, tag="xhat")
                    nc.any.tensor_copy(out=xb[:], in_=x_tm[tt][:])
                    ptl = pstr.tile([P, EC, P], BF16, tag="tr")
                    for kc in range(EC):
                        nc.tensor.transpose(ptl[:, kc, :],
                                            xb[:, kc * P : (kc + 1) * P], id_bf[:])
                    xf = tk.tile([P, EC, P], BF16, tag="xf")
                    if tt % 2 == 0:
                        nc.vector.tensor_copy(xf[:], ptl[:])
                    else:
                        nc.scalar.copy(xf[:], ptl[:])
                    pl = psmm.tile([P, GROUP], F32, tag="mm")
                    for kc in range(EC):
                        nc.tensor.matmul(pl[:, :V], xf[:, kc, :], wl_sb[:, kc, :],
                                         start=(kc == 0), stop=(kc == EC - 1))
                    lg = tk.tile([P, V], F32, tag="lg")
                    if zero_bias:
                        if tt % 2 == 0:
                            nc.scalar.copy(lg[:], pl[:, :V])
                        else:
                            nc.vector.tensor_copy(lg[:], pl[:, :V])
                    else:
                        nc.vector.tensor_tensor(lg[:], pl[:, :V], blr_sb[:], OP.add)
                    nc.sync.dma_start(out[tt * P : (tt + 1) * P, :], lg[:])

    nc.compile()
    return nc


def _prep_host(inputs):
    f32 = np.float32
    bf16 = ml_dtypes.bfloat16
    tokens = np.asarray(inputs["tokens"]).astype(np.int64)
    emb = np.asarray(inputs["emb"], dtype=f32)
    pos_enc = np.asarray(inputs["pos_enc"], dtype=f32)
    Wq = np.asarray(inputs["Wq"], dtype=f32)
    Wk = np.asarray(inputs["Wk"], dtype=f32)
    Wv = np.asarray(inputs["Wv"], dtype=f32)
    Wo = np.asarray(inputs["Wo"], dtype=f32)
    W1 = np.asarray(inputs["W1"], dtype=f32)
    W2 = np.asarray(inputs["W2"], dtype=f32)
    Wl = np.asarray(inputs["Wl"], dtype=f32)
    bq = np.asarray(inputs["bq"], dtype=f32)
    bk = np.asarray(inputs["bk"], dtype=f32)
    bv = np.asarray(inputs["bv"], dtype=f32)
    bo = np.asarray(inputs["bo"], dtype=f32)
    c1 = np.asarray(inputs["c1"], dtype=f32)
    c2 = np.asarray(inputs["c2"], dtype=f32)
    bl = np.asarray(inputs["bl"], dtype=f32)
    g1 = np.asarray(inputs["ln1_g"], dtype=f32)
    b1 = np.asarray(inputs["ln1_b"], dtype=f32)
    g2 = np.asarray(inputs["ln2_g"], dtype=f32)
    b2 = np.asarray(inputs["ln2_b"], dtype=f32)

    scale = D ** -0.5
    wq_f = np.empty((L, E, E), f32)
    wk_f = np.empty((L, E, E), f32)
    wv_f = np.empty((L, E, E), f32)
    w1_f = np.empty((L, E, DFF), f32)
    bq_f = np.empty((L, E), f32)
    bk_f = np.empty((L, E), f32)
    bv_f = np.empty((L, E), f32)
    c1_f = np.empty((L, DFF), f32)
    for l in range(L):
        wq_f[l] = g1[l][:, None] * Wq[l] * scale
        bq_f[l] = (b1[l] @ Wq[l] + bq[l]) * scale
        wk_f[l] = g1[l][:, None] * Wk[l]
        bk_f[l] = b1[l] @ Wk[l] + bk[l]
        wv_f[l] = g1[l][:, None] * Wv[l]
        bv_f[l] = b1[l] @ Wv[l] + bv[l]
        w1_f[l] = g2[l][:, None] * W1[l]
        c1_f[l] = b2[l] @ W1[l] + c1[l]

    # maskt[k, q] = 0 if k <= q else NEG  (transposed causal mask)
    maskt = np.where(np.tril(np.ones((P, P), bool)).T, 0.0, NEG).astype(bf16)

    common = {
        "embp": np.zeros((P, E), bf16),
        "pose": pos_enc,
        "maskt": maskt,
        "wq": wq_f.astype(bf16),
        "wk": wk_f.astype(bf16),
        "wv": wv_f.astype(bf16),
        "wo": Wo.astype(bf16),
        "w1": w1_f.astype(bf16),
        "w2": W2.astype(bf16),
        "wl": Wl.astype(bf16),
        "bqf": np.ascontiguousarray(bq_f.reshape(L, EC, P).transpose(0, 2, 1)),
        "bkf": np.ascontiguousarray(bk_f.reshape(L, EC, P).transpose(0, 2, 1)),
        "c1f": np.ascontiguousarray(c1_f.reshape(L, FC, P).transpose(0, 2, 1)),
        "btm": np.ascontiguousarray(
            np.broadcast_to(
                np.stack([bv_f, bo, c2], axis=1)[:, :, None, :], (L, 3, P, E)
            )
        ).astype(f32),
        "blr": np.broadcast_to(bl[None, :], (P, V)).astype(f32),
    }
    common["embp"][:V, :] = emb.astype(bf16)

    in_maps = []
    for c in range(N_CORES):
        tok_c = tokens[c * B_LOC : (c + 1) * B_LOC].reshape(-1)
        oht = np.zeros((P, NTOK), bf16)
        oht[tok_c, np.arange(NTOK)] = 1
        m = dict(common)
        m["oht"] = oht
        in_maps.append(m)
    return in_maps


def _biases_all_zero(inputs):
    zs = [inputs[k] for k in ("bq", "bk", "bv", "bo", "c1", "c2", "bl",
                              "ln1_b", "ln2_b")]
    return all(not np.any(np.asarray(z)) for z in zs)


def kernel(**inputs) -> np.ndarray:
    global _PROG
    zb = _biases_all_zero(inputs)
    if _PROG is None or _PROG[1] != zb:
        _PROG = (build_program(zero_bias=zb), zb)
    nc = _PROG[0]
    in_maps = _prep_host(inputs)
    res = run_bass_kernel_spmd(nc, in_maps, list(range(N_CORES)))
    outs = [res.results[c]["out"].reshape(B_LOC, T, V) for c in range(N_CORES)]
    return np.concatenate(outs, axis=0).astype(np.float32)
